# revision 7
# baseline (speedup 1.0000x reference)
"""GAT (2-layer graph attention network) Trainium2 Bass kernel.

Strategy (8 NeuronCores, SPMD, destination-node row-parallel):
  - Each core owns S = N/8 = 256 destination rows i.
  - Scores are laid out j-on-partitions / (head, i)-on-free so that the
    softmax-over-j reduction is obtained for free from the aggregation
    matmul (an extra ones-column in g gives the row sums), and the masked
    probability tiles are directly the matmul stationary operand - no
    transposes anywhere.
  - Per j-chunk of 128 source nodes, the rank-2 score field
    u[j,(h,i)] = er[j,h] + el[i,h] is generated by one K=9 TensorE matmul:
    stationary rows are er^T (8) + ones (1), moving rows are a
    block-diagonal head indicator (8) + el flattened (1). The tiny el/er
    vectors (x @ (W1_head @ a)) are precomputed on the host.
  - ACT does LeakyReLU (Prelu, alpha=0.2) then Exp (both live in the
    exp_and_others table set); DVE multiplies by the 0/1 adjacency mask
    (column-sliced adj^T, replicated across heads with a step-0 AP).
  - Aggregation accumulates over the 16 j-chunks into PSUM; the ones
    column yields the softmax denominator; normalization folds into
    small per-head tensor_scalar ops; ELU is computed manually
    (elu(z) = relu(z) + exp(min(z,0)) - 1).
  - Layer 2 (single head) repeats the scheme after a host-side gather of
    the per-core layer-1 outputs (two NEFF launches, no collectives).
"""

import os
import sys

sys.path.insert(0, "/opt/trn_rl_repo")
os.environ.setdefault("MYCRO_LOCAL_CACHE", "1")

import numpy as np

import concourse.bass as bass
from concourse import bacc
import concourse.mybir as mybir
import concourse.tile as tile
from concourse.bass import ds, ts

F32 = mybir.dt.float32
AF = mybir.ActivationFunctionType
ALU = mybir.AluOpType

N = 2048          # nodes
IN = 512          # input features
HID = 256         # layer-1 hidden (8 heads x 32)
OUT = 128         # layer-2 features (1 head)
H = 8             # layer-1 heads
F1 = HID // H     # 32 features/head
M = 8             # cores
S = N // M        # 256 destination rows per core
JC = N // 128     # 16 j-chunks
SLOPE = 0.2       # LeakyReLU negative slope


def _rep_heads(ap, nrep):
    """Insert a step-0 free dim of size nrep after the partition dim:
    [128, F] -> [128, nrep, F] reading the same data nrep times."""
    return bass.AP(
        tensor=ap.tensor,
        offset=ap.offset,
        ap=[ap.ap[0], [0, nrep], *ap.ap[1:]],
    )


def build_layer1():
    nc = bacc.Bacc(None, target_bir_lowering=False)
    xT = nc.dram_tensor("xT", [IN, N], F32, kind="ExternalInput")
    W1d = nc.dram_tensor("W1d", [IN, HID], F32, kind="ExternalInput")
    adjT = nc.dram_tensor("adjT", [N, S], F32, kind="ExternalInput")
    lhsTu_d = nc.dram_tensor("lhsTu_d", [H + 1, N], F32, kind="ExternalInput")
    rhsu_d = nc.dram_tensor("rhsu_d", [H + 1, H * S], F32, kind="ExternalInput")
    hout = nc.dram_tensor("hout", [S, HID], F32, kind="ExternalOutput")

    CC = IN // 128  # 4 contraction chunks

    with tile.TileContext(nc) as tc:
        with (
            tc.tile_pool(name="const", bufs=1) as const,
            tc.tile_pool(name="sb", bufs=2) as sb,
            tc.tile_pool(name="scores", bufs=2) as scores,
            tc.tile_pool(name="psum_agg", bufs=1, space="PSUM") as psum_agg,
        ):
            # ---- resident inputs ----
            xTs = const.tile([128, CC, N], F32)
            nc.sync.dma_start(out=xTs, in_=xT.rearrange("(cc p) j -> p cc j", p=128))
            W1s = const.tile([128, CC, HID], F32)
            nc.sync.dma_start(out=W1s, in_=W1d.rearrange("(cc p) f -> p cc f", p=128))
            adjt = const.tile([128, JC, S], F32)
            nc.sync.dma_start(
                out=adjt, in_=adjT.rearrange("(jc p) i -> p jc i", p=128)
            )
            lhsTu = const.tile([H + 1, N], F32)
            nc.sync.dma_start(out=lhsTu, in_=lhsTu_d[:, :])
            rhsu = const.tile([H + 1, H * S], F32)
            nc.sync.dma_start(out=rhsu, in_=rhsu_d[:, :])

            g1aug = const.tile([128, JC, H, F1 + 1], F32)
            nc.vector.memset(g1aug[:, :, :, F1 : F1 + 1], 1.0)

            # ---- phase A: g1 = x @ W1 for all nodes (j on partitions) ----
            with tc.tile_pool(name="psum_pre", bufs=2, space="PSUM") as pp:
                for jc in range(JC):
                    gx = pp.tile([128, HID], F32, tag="gx")
                    for cc in range(CC):
                        nc.tensor.matmul(
                            gx,
                            xTs[:, cc, ts(jc, 128)],
                            W1s[:, cc, :],
                            start=(cc == 0),
                            stop=(cc == CC - 1),
                        )
                    # scatter heads into 33-strided layout (ones col at 32)
                    nc.scalar.copy(
                        g1aug[:, jc, :, 0:F1],
                        gx.rearrange("p (h f) -> p h f", h=H),
                    )

            # ---- aggregation accumulators ----
            agg = [psum_agg.tile([128, H * (F1 + 1)], F32, tag=f"agg{ih}",
                                 name=f"agg{ih}") for ih in range(2)]

            # ---- main loop over 16 j-chunks ----
            with tc.tile_pool(name="psum_u", bufs=1, space="PSUM") as pu:
                for jc in range(JC):
                    ups = pu.tile([128, H * S], F32, tag="ups")
                    for nn in range(4):
                        nc.tensor.matmul(
                            ups[:, ts(nn, 512)],
                            lhsTu[:, ts(jc, 128)],
                            rhsu[:, ts(nn, 512)],
                            start=True,
                            stop=True,
                        )
                    tlr = scores.tile([128, H * S], F32, tag="tlr")
                    nc.scalar.activation(tlr, ups, AF.Prelu, alpha=SLOPE)
                    pexp = scores.tile([128, H * S], F32, tag="pexp")
                    nc.scalar.activation(pexp, tlr, AF.Exp)
                    pm = scores.tile([128, H * S], F32, tag="pm")
                    nc.vector.tensor_mul(
                        pm.rearrange("p (h i) -> p h i", h=H),
                        pexp.rearrange("p (h i) -> p h i", h=H),
                        _rep_heads(adjt[:, jc, :], H),
                    )
                    for h in range(H):
                        for ih in range(2):
                            nc.tensor.matmul(
                                agg[ih][:, ts(h, F1 + 1)],
                                pm[:, ds(h * S + ih * 128, 128)],
                                g1aug[:, jc, h, :],
                                start=(jc == 0 and h == 0),
                                stop=(jc == JC - 1 and h == H - 1),
                            )

            # ---- finalize: normalize + ELU + store ----
            for ih in range(2):
                aggv = agg[ih].rearrange("p (h f) -> p h f", h=H)
                rec = sb.tile([128, H], F32, tag="rec")
                nc.vector.reciprocal(rec, aggv[:, :, F1])
                hcat = sb.tile([128, HID], F32, tag="hcat")
                for h in range(H):
                    nc.vector.tensor_scalar_mul(
                        hcat[:, ts(h, F1)], aggv[:, h, 0:F1], rec[:, h : h + 1]
                    )
                # elu(z) = relu(z) + exp(min(z,0)) - 1
                zneg = sb.tile([128, HID], F32, tag="zneg")
                nc.vector.tensor_scalar_min(zneg, hcat, 0.0)
                ez = sb.tile([128, HID], F32, tag="ez")
                nc.scalar.activation(ez, zneg, AF.Exp)
                zpos = sb.tile([128, HID], F32, tag="zpos")
                nc.vector.tensor_scalar_max(zpos, hcat, 0.0)
                hfin = sb.tile([128, HID], F32, tag="hfin")
                nc.vector.scalar_tensor_tensor(
                    hfin, ez, -1.0, zpos, ALU.add, ALU.add
                )
                nc.sync.dma_start(out=hout[ts(ih, 128), :], in_=hfin)

    nc.finalize()
    return nc


def build_layer2():
    nc = bacc.Bacc(None, target_bir_lowering=False)
    hT = nc.dram_tensor("hT", [HID, N], F32, kind="ExternalInput")
    W2d = nc.dram_tensor("W2d", [HID, OUT], F32, kind="ExternalInput")
    adjT = nc.dram_tensor("adjT", [N, S], F32, kind="ExternalInput")
    lhsTu_d = nc.dram_tensor("lhsTu_d", [2, N], F32, kind="ExternalInput")
    rhsu_d = nc.dram_tensor("rhsu_d", [2, S], F32, kind="ExternalInput")
    out2 = nc.dram_tensor("out2", [S, OUT], F32, kind="ExternalOutput")

    CC = HID // 128  # 2 contraction chunks

    with tile.TileContext(nc) as tc:
        with (
            tc.tile_pool(name="const", bufs=1) as const,
            tc.tile_pool(name="sb", bufs=2) as sb,
            tc.tile_pool(name="scores", bufs=2) as scores,
            tc.tile_pool(name="psum_agg", bufs=1, space="PSUM") as psum_agg,
        ):
            hTs = const.tile([128, CC, N], F32)
            nc.sync.dma_start(out=hTs, in_=hT.rearrange("(cc p) j -> p cc j", p=128))
            W2s = const.tile([128, CC, OUT], F32)
            nc.sync.dma_start(out=W2s, in_=W2d.rearrange("(cc p) f -> p cc f", p=128))
            adjt = const.tile([128, JC, S], F32)
            nc.sync.dma_start(
                out=adjt, in_=adjT.rearrange("(jc p) i -> p jc i", p=128)
            )
            lhsTu = const.tile([2, N], F32)
            nc.sync.dma_start(out=lhsTu, in_=lhsTu_d[:, :])
            rhsu = const.tile([2, S], F32)
            nc.sync.dma_start(out=rhsu, in_=rhsu_d[:, :])

            g2aug = const.tile([128, JC, OUT + 1], F32)
            nc.vector.memset(g2aug[:, :, OUT : OUT + 1], 1.0)

            with tc.tile_pool(name="psum_pre", bufs=2, space="PSUM") as pp:
                for jc in range(JC):
                    gx = pp.tile([128, OUT], F32, tag="gx")
                    for cc in range(CC):
                        nc.tensor.matmul(
                            gx,
                            hTs[:, cc, ts(jc, 128)],
                            W2s[:, cc, :],
                            start=(cc == 0),
                            stop=(cc == CC - 1),
                        )
                    nc.scalar.copy(g2aug[:, jc, 0:OUT], gx)

            agg = [psum_agg.tile([128, OUT + 1], F32, tag=f"agg{ih}",
                                 name=f"agg{ih}") for ih in range(2)]

            # pairs of j-chunks share one PSUM bank to double the ACT op size
            with tc.tile_pool(name="psum_u", bufs=2, space="PSUM") as pu:
                for jcp in range(JC // 2):
                    ups = pu.tile([128, 2 * S], F32, tag="ups")
                    for half in range(2):
                        jc = 2 * jcp + half
                        nc.tensor.matmul(
                            ups[:, ts(half, S)],
                            lhsTu[:, ts(jc, 128)],
                            rhsu,
                            start=(half == 0),
                            stop=(half == 1),
                        )
                    tlr = scores.tile([128, 2 * S], F32, tag="tlr")
                    nc.scalar.activation(tlr, ups, AF.Prelu, alpha=SLOPE)
                    pexp = scores.tile([128, 2 * S], F32, tag="pexp")
                    nc.scalar.activation(pexp, tlr, AF.Exp)
                    pm = scores.tile([128, 2 * S], F32, tag="pm")
                    nc.vector.tensor_mul(
                        pm.rearrange("p (c i) -> p c i", c=2),
                        pexp.rearrange("p (c i) -> p c i", c=2),
                        adjt[:, ds(2 * jcp, 2), :],
                    )
                    for half in range(2):
                        jc = 2 * jcp + half
                        for ih in range(2):
                            nc.tensor.matmul(
                                agg[ih],
                                pm[:, ds(half * S + ih * 128, 128)],
                                g2aug[:, jc, :],
                                start=(jc == 0),
                                stop=(jc == JC - 1),
                            )

            for ih in range(2):
                rec = sb.tile([128, 1], F32, tag="rec")
                nc.vector.reciprocal(rec, agg[ih][:, OUT : OUT + 1])
                outt = sb.tile([128, OUT], F32, tag="outt")
                nc.vector.tensor_scalar_mul(outt, agg[ih][:, 0:OUT], rec)
                nc.sync.dma_start(out=out2[ts(ih, 128), :], in_=outt)

    nc.finalize()
    return nc


_programs = {}


def _get_programs():
    if "l1" not in _programs:
        _programs["l1"] = build_layer1()
        _programs["l2"] = build_layer2()
    return _programs["l1"], _programs["l2"]


def _blockdiag_el(el, heads, s):
    """rhs_u: [heads+1, heads*s]: rows 0..heads-1 = head indicator,
    last row = el flattened (h, i)."""
    r = np.zeros((heads + 1, heads * s), dtype=np.float32)
    for h in range(heads):
        r[h, h * s : (h + 1) * s] = 1.0
    r[heads, :] = np.ascontiguousarray(el.T).reshape(-1)  # [h, i] flat
    return r


def _prep_layer1_inputs(x, W1, a1_l, a1_r, adjT_f32):
    xT = np.ascontiguousarray(x.T)
    W1h = W1.reshape(IN, H, F1)
    er = x @ np.ascontiguousarray(W1h @ a1_r)        # [N, H]
    el = x @ np.ascontiguousarray(W1h @ a1_l)        # [N, H]
    lhsTu = np.concatenate(
        [np.ascontiguousarray(er.T), np.ones((1, N), np.float32)], axis=0
    )  # [9, N]
    in_maps = []
    for k in range(M):
        in_maps.append({
            "xT": xT,
            "W1d": W1,
            "adjT": np.ascontiguousarray(adjT_f32[:, k * S : (k + 1) * S]),
            "lhsTu_d": lhsTu,
            "rhsu_d": _blockdiag_el(el[k * S : (k + 1) * S, :], H, S),
        })
    return in_maps


def _prep_layer2_inputs(h_full, W2, a2_l, a2_r, adjT_f32):
    hT = np.ascontiguousarray(h_full.T)
    er = h_full @ np.ascontiguousarray(W2 @ a2_r)    # [N]
    el = h_full @ np.ascontiguousarray(W2 @ a2_l)    # [N]
    lhsTu = np.stack([er, np.ones(N, np.float32)], axis=0)  # [2, N]
    in_maps = []
    for k in range(M):
        rhsu = np.stack(
            [np.ones(S, np.float32), el[k * S : (k + 1) * S]], axis=0
        )  # [2, S]
        in_maps.append({
            "hT": hT,
            "W2d": W2,
            "adjT": np.ascontiguousarray(adjT_f32[:, k * S : (k + 1) * S]),
            "lhsTu_d": lhsTu,
            "rhsu_d": rhsu,
        })
    return in_maps


def _ensure_ntff_hook():
    """The agent image's antenv lacks axon_hooks; synthesize it and install
    the boot's ctypes NTFF hook so trace=True works. Also neuter the
    artifact upload (zero-egress sandbox)."""
    import types

    import concourse.bass_utils as bu

    bu.upload_artifacts = lambda tmpdir: tmpdir
    try:
        from antenv.axon_hooks import get_axon_ntff_profile_hook  # noqa: F401
        return
    except ImportError:
        pass
    import antenv
    import trn_agent_boot.trn_boot as tb

    mod = types.ModuleType("antenv.axon_hooks")
    state = {"hook": None}
    mod.set_axon_ntff_profile_hook = lambda h: state.__setitem__("hook", h)
    mod.get_axon_ntff_profile_hook = lambda: state["hook"]
    sys.modules["antenv.axon_hooks"] = mod
    antenv.axon_hooks = mod
    mod.set_axon_ntff_profile_hook(
        tb._ntff_profile_via_ctypes("/opt/axon/libaxon_pjrt.so")
    )


def _run(nc, in_maps, trace=False):
    from concourse.bass_utils import run_bass_kernel_spmd

    if trace:
        try:
            _ensure_ntff_hook()
        except Exception as e:  # tracing is best-effort
            print(f"ntff hook install failed: {e}")
    return run_bass_kernel_spmd(nc, in_maps, list(range(M)), trace=trace)


def kernel(x, W1, a1_l, a1_r, W2, a2_l, a2_r, adj_mat, _trace=False, _results=None):
    x = np.asarray(x, dtype=np.float32)
    W1 = np.asarray(W1, dtype=np.float32)
    a1_l = np.asarray(a1_l, dtype=np.float32)
    a1_r = np.asarray(a1_r, dtype=np.float32)
    W2 = np.asarray(W2, dtype=np.float32)
    a2_l = np.asarray(a2_l, dtype=np.float32)
    a2_r = np.asarray(a2_r, dtype=np.float32)
    adjT_f32 = np.ascontiguousarray(np.asarray(adj_mat).T.astype(np.float32))

    l1, l2 = _get_programs()

    r1 = _run(l1, _prep_layer1_inputs(x, W1, a1_l, a1_r, adjT_f32), trace=_trace)
    h_full = np.concatenate([r1.results[k]["hout"] for k in range(M)], axis=0)

    r2 = _run(l2, _prep_layer2_inputs(h_full, W2, a2_l, a2_r, adjT_f32), trace=_trace)
    out = np.concatenate([r2.results[k]["out2"] for k in range(M)], axis=0)

    if _results is not None:
        _results["r1"] = r1
        _results["r2"] = r2
        _results["h_full"] = h_full
    return out
